# revision 22
# baseline (speedup 1.0000x reference)
"""CenterLoss on Trainium2, data-parallel across 8 NeuronCores.

reference:
    distmat = 0.5*(||x||^2 + ||c||^2) + 0.3 * x @ centers.T        [B, C]
    loss = sum(clip(distmat * onehot(labels), 1e-12, 1e12)) / B

The mask keeps exactly one entry per row (j == labels[i]); every other
entry becomes clip(0) = 1e-12.  So

    loss = ( sum_i clip(d_i, 1e-12, 1e12) + B*(C-1)*1e-12 ) / B
    d_i  = 0.5*(||x_i||^2 + ||c_{l_i}||^2) + 0.3 * x_i . c_{l_i}

Per core (512 rows):
  - labels arrive via one tiny DMA on the ACT HWDGE ring (lowest-latency
    ring with no other traffic), then 4 indirect-DMA gathers fetch the
    labeled center rows tile by tile (FIFO, so tile 0 lands first);
  - x rows stream in on the SP HWDGE ring; ||x||^2 row-sums run on the
    Vector engine while the gathers are still in flight;
  - per gathered tile: ||c||^2 on ACT (Square+accum), 0.3*x.c on Vector
    (fused multiply+row-sum), then d = 0.5*(sqx+sqc) + dot, clipped;
  - a PE matmul against ones reduces partitions to the per-core scalar.
The host sums the 8 per-core scalars (the "all-reduce"), adds the clip
constant and divides by B.
"""

import os

import numpy as np

import concourse.bass as bass
import concourse.bacc as bacc
import concourse.tile as tile
from concourse import mybir
from concourse.bass_utils import run_bass_kernel_spmd

B, C, D = 4096, 10000, 512
NCORES = 8
BS = B // NCORES  # 512 rows per core
P = 128
NT = BS // P  # 4 partition-tiles per core

F32 = mybir.dt.float32
I32 = mybir.dt.int32

# Debug hooks for test.py (harness just calls kernel()).
TRACE = os.environ.get("KERNEL_TRACE", "0") == "1"
LAST_RESULTS = [None]

_NC_CACHE = []


def _build_bass():
    # Bacc (not raw Bass): its finalize() legalizes multi-wait instructions
    # into event semaphores, which walrus codegen requires.
    nc = bacc.Bacc(None, target_bir_lowering=False, num_swdge_queues=4)
    x_in = nc.dram_tensor("x", [BS, D], F32, kind="ExternalInput")
    lab_in = nc.dram_tensor("labels", [BS], I32, kind="ExternalInput")
    cen_in = nc.dram_tensor("centers", [C, D], F32, kind="ExternalInput")
    out_t = nc.dram_tensor("out", [P, 1], F32, kind="ExternalOutput")

    # shard row n*128+p lives at partition p, slot n
    x_pnd = x_in.rearrange("(n p) d -> p n d", p=P)
    lab_pn = lab_in.rearrange("(n p) -> p n", p=P)

    with tile.TileContext(nc) as tc:
        with (
            tc.tile_pool(name="io", bufs=1) as io,
            tc.tile_pool(name="scratch", bufs=2) as scratch,
            tc.tile_pool(name="accp", bufs=1) as accp,
        ):
            acc = accp.tile([P, NT], F32)

            # warm the cold SWDGE ring with a tiny data-independent DMA so the
            # first gather's packets start draining ~1µs earlier
            warm = accp.tile([1, P], F32)
            nc.gpsimd.dma_start(out=warm[:, :], in_=cen_in[0:1, 0:P])

            # all 4 index columns in one tiny HWDGE DMA, issued first: the
            # gathers depend on it (measured: SP ring beats the ACT ring here,
            # the ACT ring start is delayed by the activation table load)
            idx = io.tile([P, NT], I32)
            nc.sync.dma_start(out=idx[:, :], in_=lab_pn[:, :])

            cs = [io.tile([P, D], F32, name=f"c{k}", tag=f"c{k}") for k in range(NT)]
            xs = [io.tile([P, D], F32, name=f"x{k}", tag=f"x{k}") for k in range(NT)]
            for k in range(NT):
                # FIFO on the SWDGE queue: tile k's rows complete before k+1's
                gi = nc.gpsimd.indirect_dma_start(
                    out=cs[k][:, :],
                    out_offset=None,
                    in_=cen_in[:],
                    in_offset=bass.IndirectOffsetOnAxis(ap=idx[:, k : k + 1], axis=0),
                )
                gi.ins.queue = f"qPoolDynamic{k or ''}"
            for k in range(NT):
                nc.sync.dma_start(out=xs[k][:, :], in_=x_pnd[:, k, :])

            # per-row partial sums land in columns of [P, NT] tiles so the
            # final combine runs as a few [P, NT]-wide ops instead of a chain
            # of 4x per-tile scalar ops
            sqx_all = accp.tile([P, NT], F32)
            sqc_all = accp.tile([P, NT], F32)
            dot_all = accp.tile([P, NT], F32)

            # ||x||^2 on Vector, hidden under the gather wait
            for k in range(NT):
                xx = scratch.tile([P, D], F32, tag="xx")
                nc.vector.scalar_tensor_tensor(
                    out=xx[:],
                    in0=xs[k][:, :],
                    scalar=1.0,
                    in1=xs[k][:, :],
                    op0=mybir.AluOpType.mult,
                    op1=mybir.AluOpType.mult,
                    accum_out=sqx_all[:, k : k + 1],
                )

            for k in range(NT):
                # sqc = sum(c^2) per row on ACT
                cc = scratch.tile([P, D], F32, tag="cc")
                nc.scalar.activation(
                    out=cc[:],
                    in_=cs[k][:, :],
                    func=mybir.ActivationFunctionType.Square,
                    accum_out=sqc_all[:, k : k + 1],
                )

                # dot = sum(0.3*x * c) per row on Vector
                prod = scratch.tile([P, D], F32, tag="prod")
                nc.vector.scalar_tensor_tensor(
                    out=prod[:],
                    in0=xs[k][:, :],
                    scalar=0.3,
                    in1=cs[k][:, :],
                    op0=mybir.AluOpType.mult,
                    op1=mybir.AluOpType.mult,
                    accum_out=dot_all[:, k : k + 1],
                )

            # d = 0.5*(sqx + sqc) + dot for all tiles at once, then clip
            u = accp.tile([P, NT], F32)
            nc.vector.tensor_add(out=u[:], in0=sqx_all[:], in1=sqc_all[:])
            dall = accp.tile([P, NT], F32)
            nc.vector.scalar_tensor_tensor(
                out=dall[:],
                in0=u[:],
                scalar=0.5,
                in1=dot_all[:],
                op0=mybir.AluOpType.mult,
                op1=mybir.AluOpType.add,
            )
            nc.vector.tensor_scalar(
                out=acc[:, :],
                in0=dall[:],
                scalar1=1e-12,
                scalar2=1e12,
                op0=mybir.AluOpType.max,
                op1=mybir.AluOpType.min,
            )

            # per-partition partial sums; the cross-partition sum is part of
            # the host-side unshard/all-reduce (1024 scalars across 8 cores)
            accs = accp.tile([P, 1], F32)
            nc.vector.reduce_sum(out=accs[:], in_=acc[:], axis=mybir.AxisListType.X)
            nc.sync.dma_start(out=out_t[:, :], in_=accs[:])
    nc.finalize()
    return nc


def _get_nc():
    if not _NC_CACHE:
        _NC_CACHE.append(_build_bass())
    return _NC_CACHE[0]


def kernel(x, centers, labels):
    x = np.ascontiguousarray(np.asarray(x), dtype=np.float32)
    centers = np.ascontiguousarray(np.asarray(centers), dtype=np.float32)
    labels = np.ascontiguousarray(np.asarray(labels).astype(np.int32))
    assert x.shape == (B, D) and centers.shape == (C, D) and labels.shape == (B,)

    nc = _get_nc()
    in_maps = [
        {
            "x": x[c * BS : (c + 1) * BS],
            "labels": labels[c * BS : (c + 1) * BS],
            "centers": centers,
        }
        for c in range(NCORES)
    ]
    res = run_bass_kernel_spmd(nc, in_maps, core_ids=list(range(NCORES)), trace=TRACE)
    LAST_RESULTS[0] = res

    total = float(np.sum(np.array([r["out"][:, 0] for r in res.results], np.float64)))
    total += B * (C - 1) * 1e-12
    return np.array(total / B, dtype=np.float32)


# revision 26
# speedup vs baseline: 1.2703x; 1.2703x over previous
"""CenterLoss on Trainium2, data-parallel across 8 NeuronCores.

reference:
    distmat = 0.5*(||x||^2 + ||c||^2) + 0.3 * x @ centers.T        [B, C]
    loss = sum(clip(distmat * onehot(labels), 1e-12, 1e12)) / B

The mask keeps exactly one entry per row (j == labels[i]); every other
entry becomes clip(0) = 1e-12.  So

    loss = ( sum_i clip(d_i, 1e-12, 1e12) + B*(C-1)*1e-12 ) / B
    d_i  = 0.5*(||x_i||^2 + ||c_{l_i}||^2) + 0.3 * x_i . c_{l_i}

Per core (512 rows):
  - labels arrive via one tiny DMA on the ACT HWDGE ring (lowest-latency
    ring with no other traffic), then 4 indirect-DMA gathers fetch the
    labeled center rows tile by tile (FIFO, so tile 0 lands first);
  - x rows stream in on the SP HWDGE ring; ||x||^2 row-sums run on the
    Vector engine while the gathers are still in flight;
  - per gathered tile: ||c||^2 on ACT (Square+accum), 0.3*x.c on Vector
    (fused multiply+row-sum), then d = 0.5*(sqx+sqc) + dot, clipped;
  - a PE matmul against ones reduces partitions to the per-core scalar.
The host sums the 8 per-core scalars (the "all-reduce"), adds the clip
constant and divides by B.
"""

import os

import numpy as np

import concourse.bass as bass
import concourse.bacc as bacc
import concourse.tile as tile
from concourse import mybir
from concourse.bass_utils import run_bass_kernel_spmd

B, C, D = 4096, 10000, 512
NCORES = 8
BS = B // NCORES  # 512 rows per core
P = 128
NT = BS // P  # 4 partition-tiles per core

F32 = mybir.dt.float32
I32 = mybir.dt.int32

# Debug hooks for test.py (harness just calls kernel()).
TRACE = os.environ.get("KERNEL_TRACE", "0") == "1"
LAST_RESULTS = [None]

_NC_CACHE = []


def _build_bass():
    # Bacc (not raw Bass): its finalize() legalizes multi-wait instructions
    # into event semaphores, which walrus codegen requires.
    nc = bacc.Bacc(None, target_bir_lowering=False, num_swdge_queues=4)
    x_in = nc.dram_tensor("x", [BS, D], F32, kind="ExternalInput")
    lab_in = nc.dram_tensor("labels", [BS], I32, kind="ExternalInput")
    cen_in = nc.dram_tensor("centers", [C, D], F32, kind="ExternalInput")
    out_t = nc.dram_tensor("out", [1, 1], F32, kind="ExternalOutput")

    # shard row n*128+p lives at partition p, slot n
    x_pnd = x_in.rearrange("(n p) d -> p n d", p=P)
    lab_pn = lab_in.rearrange("(n p) -> p n", p=P)

    with tile.TileContext(nc) as tc:
        with (
            tc.tile_pool(name="io", bufs=1) as io,
            tc.tile_pool(name="scratch", bufs=2) as scratch,
            tc.tile_pool(name="accp", bufs=1) as accp,
            tc.tile_pool(name="psum", bufs=1, space="PSUM") as psum,
        ):
            acc = accp.tile([P, NT], F32)
            ones = accp.tile([P, 1], F32)
            nc.vector.memset(ones[:], 1.0)

            # all 4 index columns in one tiny HWDGE DMA, issued first: the
            # gathers depend on it (measured: SP ring beats the ACT ring here,
            # the ACT ring start is delayed by the activation table load)
            idx = io.tile([P, NT], I32)
            nc.sync.dma_start(out=idx[:, :], in_=lab_pn[:, :])

            cs = [io.tile([P, D], F32, name=f"c{k}", tag=f"c{k}") for k in range(NT)]
            xs = [io.tile([P, D], F32, name=f"x{k}", tag=f"x{k}") for k in range(NT)]
            for k in range(NT):
                # FIFO on the SWDGE queue: tile k's rows complete before k+1's
                gi = nc.gpsimd.indirect_dma_start(
                    out=cs[k][:, :],
                    out_offset=None,
                    in_=cen_in[:],
                    in_offset=bass.IndirectOffsetOnAxis(ap=idx[:, k : k + 1], axis=0),
                )
                gi.ins.queue = f"qPoolDynamic{k or ''}"
            for k in range(NT):
                nc.sync.dma_start(out=xs[k][:, :], in_=x_pnd[:, k, :])

            # per-row partial sums land in columns of [P, NT] tiles so the
            # final combine runs as a few [P, NT]-wide ops instead of a chain
            # of 4x per-tile scalar ops
            sqx_all = accp.tile([P, NT], F32)
            sqc_all = accp.tile([P, NT], F32)
            dot_all = accp.tile([P, NT], F32)

            # ||x||^2 on Vector, hidden under the gather wait
            for k in range(NT):
                xx = scratch.tile([P, D], F32, tag="xx")
                nc.vector.scalar_tensor_tensor(
                    out=xx[:],
                    in0=xs[k][:, :],
                    scalar=1.0,
                    in1=xs[k][:, :],
                    op0=mybir.AluOpType.mult,
                    op1=mybir.AluOpType.mult,
                    accum_out=sqx_all[:, k : k + 1],
                )

            pre_all = accp.tile([P, NT], F32)
            for k in range(NT):
                # sqc = sum(c^2) per row on ACT
                cc = scratch.tile([P, D], F32, tag="cc")
                nc.scalar.activation(
                    out=cc[:],
                    in_=cs[k][:, :],
                    func=mybir.ActivationFunctionType.Square,
                    accum_out=sqc_all[:, k : k + 1],
                )

                # dot = sum(0.3*x * c) per row on Vector
                prod = scratch.tile([P, D], F32, tag="prod")
                nc.vector.scalar_tensor_tensor(
                    out=prod[:],
                    in0=xs[k][:, :],
                    scalar=0.3,
                    in1=cs[k][:, :],
                    op0=mybir.AluOpType.mult,
                    op1=mybir.AluOpType.mult,
                    accum_out=dot_all[:, k : k + 1],
                )

                # pre = 0.5*sqx + dot, hidden under the gather/ACT pipeline;
                # only tile k's column is touched so later tiles don't stall
                nc.vector.scalar_tensor_tensor(
                    out=pre_all[:, k : k + 1],
                    in0=sqx_all[:, k : k + 1],
                    scalar=0.5,
                    in1=dot_all[:, k : k + 1],
                    op0=mybir.AluOpType.mult,
                    op1=mybir.AluOpType.add,
                )

            # d = 0.5*sqc + pre for all tiles at once, then clip
            dall = accp.tile([P, NT], F32)
            nc.vector.scalar_tensor_tensor(
                out=dall[:],
                in0=sqc_all[:],
                scalar=0.5,
                in1=pre_all[:],
                op0=mybir.AluOpType.mult,
                op1=mybir.AluOpType.add,
            )
            nc.vector.tensor_scalar(
                out=acc[:, :],
                in0=dall[:],
                scalar1=1e-12,
                scalar2=1e12,
                op0=mybir.AluOpType.max,
                op1=mybir.AluOpType.min,
            )

            accs = accp.tile([P, 1], F32)
            nc.vector.reduce_sum(out=accs[:], in_=acc[:], axis=mybir.AxisListType.X)

            # partition reduce on PE: one [1,1] result = one out-DMA descriptor
            # (a [128,1] output would be 128 tiny packets, ~11us of drain)
            ps = psum.tile([1, 1], F32)
            nc.tensor.matmul(out=ps[:], lhsT=accs[:], rhs=ones[:], start=True, stop=True)
            res = accp.tile([1, 1], F32)
            nc.vector.tensor_copy(out=res[:], in_=ps[:])
            nc.sync.dma_start(out=out_t[:, :], in_=res[:])
    nc.finalize()
    return nc


def _get_nc():
    if not _NC_CACHE:
        _NC_CACHE.append(_build_bass())
    return _NC_CACHE[0]


def kernel(x, centers, labels):
    x = np.ascontiguousarray(np.asarray(x), dtype=np.float32)
    centers = np.ascontiguousarray(np.asarray(centers), dtype=np.float32)
    labels = np.ascontiguousarray(np.asarray(labels).astype(np.int32))
    assert x.shape == (B, D) and centers.shape == (C, D) and labels.shape == (B,)

    nc = _get_nc()
    in_maps = [
        {
            "x": x[c * BS : (c + 1) * BS],
            "labels": labels[c * BS : (c + 1) * BS],
            "centers": centers,
        }
        for c in range(NCORES)
    ]
    res = run_bass_kernel_spmd(nc, in_maps, core_ids=list(range(NCORES)), trace=TRACE)
    LAST_RESULTS[0] = res

    total = float(np.sum(np.array([r["out"][0, 0] for r in res.results], np.float64)))
    total += B * (C - 1) * 1e-12
    return np.array(total / B, dtype=np.float32)
